# revision 5
# baseline (speedup 1.0000x reference)
"""Trainium2 Bass kernel for nn_KANStressPredictor (planar bf16, 8-core DP).

Math per strain triple (s0, s1, s2), with C = 2E + I symmetric 2x2:
    t12 = (s0+s1) -/+ rad,  rad = sqrt((s0-s1)^2 + s2^2)
    l_i = ln(t_i + 1)                       (eigenvalues are t_i + 1)
    out_i = exp(ki0/3 * (l_i - 0.5*l_other))     i in {0,1}
    out_2 = ki1 * 0.5 * (l1 + l2)

Implementation notes:
  * bf16 end-to-end (host casts): halves HBM traffic vs f32 and unlocks
    DVE 2x (tensor_tensor) / 4x (tensor_scalar) perf modes.
  * Planar per-chunk layout [a|b|c] per partition row (host pre-transposes)
    so every engine op is a dense step-1 slice; strided access would force
    the DVE into 1x mode.
  * rad via exp(0.5*ln(r2)) keeps all activations in the single
    natural_log_exp_and_others table set; one explicit LoadActFuncSet up
    front means zero table reloads (a greedy chooser otherwise ping-pongs
    exp_and_others/natural_log every chunk, ~2.7us per reload).
  * No scalar_tensor_tensor (no DVE accel uops - always 1x).  The affine
    combos are restructured as h = 0.5*l12 (one 4x tensor_scalar), then
    w_i = l_i - h_other and out2 = h1 + h2 as plain 2x tensor_tensors.
  * GPSIMD (Pool) takes c^2 and out2 off the DVE.
  * Both DMA streams issue from the SP sequencer (qSPDynamicHW).  Routing
    the out-DMAs onto the ACT ring (OUT_SC) was tried and measured no
    better: the out-DMA's semaphore waits sit in the ACT instruction
    stream and can stall activation compute.

Sharding: pure data-parallel over the batch dim across 8 cores; host
reassembles.  ki0/ki1 are compile-time constants (cached per value).
"""

import sys

for _p in ("/opt/trn_rl_repo",):
    if _p not in sys.path:
        sys.path.insert(0, _p)

import numpy as np
import ml_dtypes

import concourse.bacc as bacc
import concourse.bass as bass
import concourse.tile as tile
from concourse import mybir
from concourse.bass_utils import run_bass_kernel_spmd

N_CORES = 8
P = 128
BF16 = ml_dtypes.bfloat16
FP8 = ml_dtypes.float8_e4m3
# Input rides as float8_e4m3 pre-scaled by FP8_SCALE; the scale flows
# linearly through s/u/rad/t and is divided out for free by the Ln
# activation's input-scale.  Measured rel err 1.07e-2 (gate 2e-2).
FP8_SCALE = 32.0
IN_FP8 = True

# Tuned on hardware (reps-marginal benchmarks):
CT_DEFAULT = 1024      # triples per chunk -> per-partition chunk [a|b|c]
IO_BUFS = 4
WK_BUFS = 3
OUT_SC = False         # out-DMA ring: False = qSPDynamicHW (measured best)

_cache: dict = {}


def _lnexp_set_id(nc) -> int:
    try:
        from concourse.hw_specs import get_activation_tables

        return list(get_activation_tables(nc.m.arch)).index(
            "natural_log_exp_and_others"
        )
    except Exception:
        return 6


def _build(ki0: float, ki1: float, F: int, CT: int, reps: int = 1,
           out_sc: bool = OUT_SC, in_fp8: bool = IN_FP8):
    key = (ki0, ki1, F, CT, reps, out_sc, in_fp8)
    if key in _cache:
        return _cache[key]

    bf16 = mybir.dt.bfloat16
    in_dt = mybir.dt.float8e4 if in_fp8 else bf16
    ln_scale = 1.0 / FP8_SCALE if in_fp8 else 1.0
    AF = mybir.ActivationFunctionType
    CE = 3 * CT
    assert F % CE == 0
    n_chunks = F // CE

    nc = bacc.Bacc("TRN2", target_bir_lowering=False, debug=False)
    in_ap = nc.dram_tensor("strain", [P, F], in_dt, kind="ExternalInput").ap()
    out_ap = nc.dram_tensor("out", [P, F], bf16, kind="ExternalOutput").ap()

    nc.scalar.add_instruction(
        mybir.InstLoadActFuncSet(
            name=nc.get_next_instruction_name(),
            act_func_set_id=_lnexp_set_id(nc),
            engine=mybir.EngineType.Activation,
        )
    )

    with tile.TileContext(nc) as tc:
        with (
            tc.tile_pool(name="io", bufs=IO_BUFS) as iop,
            tc.tile_pool(name="wk", bufs=WK_BUFS) as wk,
        ):
            for ci in range(n_chunks * reps):
                ci = ci % n_chunks
                sl = bass.ts(ci, CE)
                out_eng = nc.scalar if out_sc else nc.sync
                I = iop.tile([P, CE], in_dt, name="in", tag="in")
                nc.sync.dma_start(I[:], in_ap[:, sl])
                a, b, c = I[:, 0:CT], I[:, CT : 2 * CT], I[:, 2 * CT : 3 * CT]

                s = wk.tile([P, CT], bf16, name="s", tag="s")[:]
                u = wk.tile([P, CT], bf16, name="u", tag="u")[:]
                c2 = wk.tile([P, CT], bf16, name="c2", tag="c2")[:]
                rad = wk.tile([P, CT], bf16, name="rad", tag="rad")[:]

                nc.vector.tensor_add(s, a, b)        # s0+s1
                nc.vector.tensor_sub(u, a, b)        # s0-s1
                nc.gpsimd.tensor_mul(c2, c, c)       # s2^2   (Pool)
                nc.vector.tensor_mul(u, u, u)        # (s0-s1)^2, in place
                nc.vector.tensor_add(c2, u, c2)      # r2, in place
                nc.scalar.activation(c2, c2, AF.Ln)              # ln(r2)
                nc.scalar.activation(rad, c2, AF.Exp, scale=0.5)  # rad

                T12 = wk.tile([P, 2 * CT], bf16, name="t12", tag="t12")[:]
                nc.vector.tensor_sub(T12[:, 0:CT], s, rad)   # t1
                nc.vector.tensor_add(T12[:, CT:], s, rad)    # t2
                L12 = wk.tile([P, 2 * CT], bf16, name="l12", tag="l12")[:]
                nc.scalar.activation(
                    L12, T12, AF.Ln, bias=1.0, scale=ln_scale
                )  # ln(t/SC + 1)
                H = wk.tile([P, 2 * CT], bf16, name="h", tag="h")[:]
                nc.vector.tensor_scalar_mul(H, L12, 0.5)
                l1, l2 = L12[:, 0:CT], L12[:, CT:]
                h1, h2 = H[:, 0:CT], H[:, CT:]

                O = iop.tile([P, CE], bf16, name="out", tag="out")
                W12 = T12  # reuse
                nc.vector.tensor_sub(W12[:, 0:CT], l1, h2)   # w1
                nc.vector.tensor_sub(W12[:, CT:], l2, h1)    # w2
                nc.scalar.activation(
                    O[:, 0 : 2 * CT], W12, AF.Exp, scale=ki0 / 3.0
                )  # out0, out1
                o2 = O[:, 2 * CT : 3 * CT]
                nc.gpsimd.tensor_add(o2, h1, h2)             # out2 (Pool)
                if ki1 != 1.0:
                    nc.vector.tensor_scalar_mul(o2, o2, ki1)

                out_eng.dma_start(out_ap[:, sl], O[:])

    nc.compile()
    _cache[key] = nc
    return nc


def _prep(strain: np.ndarray, CT: int, in_fp8: bool = IN_FP8) -> np.ndarray:
    """[B, T, 3] f32 -> [N_CORES, P, F] bf16/fp8 planar chunks."""
    B, T, C = strain.shape
    F = B * T * C // (N_CORES * P)
    n_chunks = F // (3 * CT)
    x = np.ascontiguousarray(strain, dtype=np.float32)
    x = (x * FP8_SCALE).astype(FP8) if in_fp8 else x.astype(BF16)
    x = x.reshape(N_CORES, P, n_chunks, CT, 3)
    x = x.transpose(0, 1, 2, 4, 3)
    return np.ascontiguousarray(x).reshape(N_CORES, P, F)


def _unprep(out: np.ndarray, B: int, T: int, CT: int) -> np.ndarray:
    F = out.shape[-1]
    n_chunks = F // (3 * CT)
    y = out.reshape(N_CORES, P, n_chunks, 3, CT)
    y = y.transpose(0, 1, 2, 4, 3)
    return np.ascontiguousarray(y).astype(np.float32).reshape(B, T, 3)


def _run(strain: np.ndarray, ki0: float, ki1: float, trace: bool = False,
         CT: int = CT_DEFAULT):
    B, T, C = strain.shape
    assert C == 3 and B % N_CORES == 0
    F = (B // N_CORES) * T * C // P
    assert F % (3 * CT) == 0

    nc = _build(float(ki0), float(ki1), F, CT)
    flat = _prep(strain, CT)
    in_maps = [{"strain": flat[i]} for i in range(N_CORES)]
    res = run_bass_kernel_spmd(nc, in_maps, list(range(N_CORES)), trace=trace)
    out = np.stack([np.asarray(res.results[i]["out"]) for i in range(N_CORES)])
    return _unprep(out, B, T, CT), res


def kernel(strain: np.ndarray, ki0, ki1) -> np.ndarray:
    out, _ = _run(
        np.asarray(strain), float(np.asarray(ki0)), float(np.asarray(ki1))
    )
    return out


# revision 7
# speedup vs baseline: 1.1204x; 1.1204x over previous
"""Trainium2 Bass kernel for nn_KANStressPredictor (planar bf16, 8-core DP).

Math per strain triple (s0, s1, s2), with C = 2E + I symmetric 2x2:
    t12 = (s0+s1) -/+ rad,  rad = sqrt((s0-s1)^2 + s2^2)
    l_i = ln(t_i + 1)                       (eigenvalues are t_i + 1)
    out_i = exp(ki0/3 * (l_i - 0.5*l_other))     i in {0,1}
    out_2 = ki1 * 0.5 * (l1 + l2)

Implementation notes:
  * The kernel is HBM-bound, so dtypes are chosen per stream: input rides
    as float8_e4m3 (host casts, x32 pre-scale divided back out for free by
    the Ln activation's input scale), output and intermediates are bf16.
    Traffic drops 25.2 MB/core (f32) -> 9.45 MB/core, and bf16 unlocks the
    DVE 2x (tensor_tensor) / 4x (tensor_scalar) perf modes.
  * Planar per-chunk layout [a|b|c] per partition row (host pre-transposes)
    so every engine op is a dense step-1 slice; strided access would force
    the DVE into 1x mode.
  * rad via exp(0.5*ln(r2)) keeps all activations in the single
    natural_log_exp_and_others table set; one explicit LoadActFuncSet up
    front means zero table reloads (a greedy chooser otherwise ping-pongs
    exp_and_others/natural_log every chunk, ~2.7us per reload).
  * No scalar_tensor_tensor (no DVE accel uops - always 1x).  The affine
    combos are restructured as h = 0.5*l12 (one 4x tensor_scalar), then
    w_i = l_i - h_other and out2 = h1 + h2 as plain 2x tensor_tensors.
  * GPSIMD (Pool) takes c^2 and out2 off the DVE.
  * Both DMA streams issue from the SP sequencer (qSPDynamicHW).  Routing
    the out-DMAs onto the ACT ring (OUT_SC) was tried and measured no
    better: the out-DMA's semaphore waits sit in the ACT instruction
    stream and can stall activation compute.

Sharding: pure data-parallel over the batch dim across 8 cores; host
reassembles.  ki0/ki1 are compile-time constants (cached per value).
"""

import sys

for _p in ("/opt/trn_rl_repo",):
    if _p not in sys.path:
        sys.path.insert(0, _p)

import numpy as np
import ml_dtypes

import concourse.bacc as bacc
import concourse.bass as bass
import concourse.tile as tile
from concourse import mybir
from concourse.bass_utils import run_bass_kernel_spmd

N_CORES = 8
P = 128
BF16 = ml_dtypes.bfloat16
FP8 = ml_dtypes.float8_e4m3
# Input rides as float8_e4m3 pre-scaled by FP8_SCALE; the scale flows
# linearly through s/u/rad/t and is divided out for free by the Ln
# activation's input-scale.  Measured rel err 1.07e-2 (gate 2e-2).
FP8_SCALE = 32.0
IN_FP8 = True

# Tuned on hardware (reps-marginal benchmarks):
CT_DEFAULT = 1024      # triples per chunk -> per-partition chunk [a|b|c]
IO_BUFS = 4
WK_BUFS = 3
OUT_SC = False         # out-DMA ring: False = qSPDynamicHW (measured best)

_cache: dict = {}


def _lnexp_set_id(nc) -> int:
    try:
        from concourse.hw_specs import get_activation_tables

        return list(get_activation_tables(nc.m.arch)).index(
            "natural_log_exp_and_others"
        )
    except Exception:
        return 6


def _build(ki0: float, ki1: float, F: int, CT: int, reps: int = 1,
           out_sc: bool = OUT_SC, in_fp8: bool = IN_FP8):
    key = (ki0, ki1, F, CT, reps, out_sc, in_fp8)
    if key in _cache:
        return _cache[key]

    bf16 = mybir.dt.bfloat16
    in_dt = mybir.dt.float8e4 if in_fp8 else bf16
    ln_scale = 1.0 / FP8_SCALE if in_fp8 else 1.0
    AF = mybir.ActivationFunctionType
    CE = 3 * CT
    assert F % CE == 0
    n_chunks = F // CE

    nc = bacc.Bacc("TRN2", target_bir_lowering=False, debug=False)
    in_ap = nc.dram_tensor("strain", [P, F], in_dt, kind="ExternalInput").ap()
    out_ap = nc.dram_tensor("out", [P, F], bf16, kind="ExternalOutput").ap()

    nc.scalar.add_instruction(
        mybir.InstLoadActFuncSet(
            name=nc.get_next_instruction_name(),
            act_func_set_id=_lnexp_set_id(nc),
            engine=mybir.EngineType.Activation,
        )
    )

    with tile.TileContext(nc) as tc:
        with (
            tc.tile_pool(name="io", bufs=IO_BUFS) as iop,
            tc.tile_pool(name="wk", bufs=WK_BUFS) as wk,
        ):
            for ci in range(n_chunks * reps):
                ci = ci % n_chunks
                sl = bass.ts(ci, CE)
                out_eng = nc.scalar if out_sc else nc.sync
                I = iop.tile([P, CE], in_dt, name="in", tag="in")
                nc.sync.dma_start(I[:], in_ap[:, sl])
                a, b, c = I[:, 0:CT], I[:, CT : 2 * CT], I[:, 2 * CT : 3 * CT]

                s = wk.tile([P, CT], bf16, name="s", tag="s")[:]
                u = wk.tile([P, CT], bf16, name="u", tag="u")[:]
                c2 = wk.tile([P, CT], bf16, name="c2", tag="c2")[:]
                rad = wk.tile([P, CT], bf16, name="rad", tag="rad")[:]

                if in_fp8:
                    # fp8 operands cap DVE tensor_tensor at 1x (2-byte dtype
                    # needed for 2x_1p); shifting s to the Pool engine keeps
                    # the DVE stream on 2x bf16 ops.  u stays on DVE - both
                    # on Pool overloads the chain head.
                    nc.gpsimd.tensor_add(s, a, b)    # s0+s1   (Pool)
                else:
                    nc.vector.tensor_add(s, a, b)    # s0+s1
                nc.vector.tensor_sub(u, a, b)        # s0-s1
                nc.gpsimd.tensor_mul(c2, c, c)       # s2^2   (Pool)
                nc.vector.tensor_mul(u, u, u)        # (s0-s1)^2, in place
                nc.vector.tensor_add(c2, u, c2)      # r2, in place
                nc.scalar.activation(c2, c2, AF.Ln)              # ln(r2)
                nc.scalar.activation(rad, c2, AF.Exp, scale=0.5)  # rad

                T12 = wk.tile([P, 2 * CT], bf16, name="t12", tag="t12")[:]
                nc.vector.tensor_sub(T12[:, 0:CT], s, rad)   # t1
                nc.vector.tensor_add(T12[:, CT:], s, rad)    # t2
                L12 = wk.tile([P, 2 * CT], bf16, name="l12", tag="l12")[:]
                nc.scalar.activation(
                    L12, T12, AF.Ln, bias=1.0, scale=ln_scale
                )  # ln(t/SC + 1)
                H = wk.tile([P, 2 * CT], bf16, name="h", tag="h")[:]
                nc.vector.tensor_scalar_mul(H, L12, 0.5)
                l1, l2 = L12[:, 0:CT], L12[:, CT:]
                h1, h2 = H[:, 0:CT], H[:, CT:]

                O = iop.tile([P, CE], bf16, name="out", tag="out")
                W12 = T12  # reuse
                nc.vector.tensor_sub(W12[:, 0:CT], l1, h2)   # w1
                nc.vector.tensor_sub(W12[:, CT:], l2, h1)    # w2
                nc.scalar.activation(
                    O[:, 0 : 2 * CT], W12, AF.Exp, scale=ki0 / 3.0
                )  # out0, out1
                o2 = O[:, 2 * CT : 3 * CT]
                nc.gpsimd.tensor_add(o2, h1, h2)             # out2 (Pool)
                if ki1 != 1.0:
                    nc.vector.tensor_scalar_mul(o2, o2, ki1)

                out_eng.dma_start(out_ap[:, sl], O[:])

    nc.compile()
    _cache[key] = nc
    return nc


def _prep(strain: np.ndarray, CT: int, in_fp8: bool = IN_FP8) -> np.ndarray:
    """[B, T, 3] f32 -> [N_CORES, P, F] bf16/fp8 planar chunks."""
    B, T, C = strain.shape
    F = B * T * C // (N_CORES * P)
    n_chunks = F // (3 * CT)
    x = np.ascontiguousarray(strain, dtype=np.float32)
    x = (x * FP8_SCALE).astype(FP8) if in_fp8 else x.astype(BF16)
    x = x.reshape(N_CORES, P, n_chunks, CT, 3)
    x = x.transpose(0, 1, 2, 4, 3)
    return np.ascontiguousarray(x).reshape(N_CORES, P, F)


def _unprep(out: np.ndarray, B: int, T: int, CT: int) -> np.ndarray:
    F = out.shape[-1]
    n_chunks = F // (3 * CT)
    y = out.reshape(N_CORES, P, n_chunks, 3, CT)
    y = y.transpose(0, 1, 2, 4, 3)
    return np.ascontiguousarray(y).astype(np.float32).reshape(B, T, 3)


def _run(strain: np.ndarray, ki0: float, ki1: float, trace: bool = False,
         CT: int = CT_DEFAULT):
    B, T, C = strain.shape
    assert C == 3 and B % N_CORES == 0
    F = (B // N_CORES) * T * C // P
    assert F % (3 * CT) == 0

    nc = _build(float(ki0), float(ki1), F, CT)
    flat = _prep(strain, CT)
    in_maps = [{"strain": flat[i]} for i in range(N_CORES)]
    res = run_bass_kernel_spmd(nc, in_maps, list(range(N_CORES)), trace=trace)
    out = np.stack([np.asarray(res.results[i]["out"]) for i in range(N_CORES)])
    return _unprep(out, B, T, CT), res


def kernel(strain: np.ndarray, ki0, ki1) -> np.ndarray:
    out, _ = _run(
        np.asarray(strain), float(np.asarray(ki0)), float(np.asarray(ki1))
    )
    return out
